# revision 15
# baseline (speedup 1.0000x reference)
"""Single-head masked attention (B=4, S=2048, D=1024, fp32) on 8 TRN2 NeuronCores.

Sharding: core c handles batch b=c//2, query half h=c%2 (1024 queries), with
K/V over the batch's UNMASKED keys only (masked keys have exactly-zero
attention weight, so they are dropped on the host). Keys are compacted and
zero-padded to K_pad = ceil(max_cnt/128)*128 (~1152 for a ~50% mask); pad
rows carry a -30000 mask bias so exp underflows to exact 0.

Matmul-work reductions vs the naive pipeline (per-core MACs 7.52G -> 4.57G):

1) scores^T = K Q^T = x (Wk^T Wq) xq^T + x (Wk^T bq)   [bk dropped: softmax
   shift invariance]. A = Wk^T Wq and c = Wk^T bq are DATA-INDEPENDENT and
   folded on the host (fp64), so the Q-projection stage disappears:
     G[d,q] = A @ xq^T + c  (one 1024^3 matmul), then S^T = x_keys @ G.
2) Key compaction: S^T, sumexp and Z contract over ~1152 instead of 2048 keys.
3) out = attnU @ (x Wv^T) / sumexp + bv = (attnU @ x_keys) Wv^T / sumexp + bv
   (V never materialized; bv exact via softmax weights summing to 1).

All matmul operands are bf16 (the compiler rejects mixed 32/16-bit operands,
NCC_IBIR034); PSUM accumulation stays fp32; output is stored bf16 and
upcast on the host. Measured end-to-end rel err ~6.5e-3 vs the 2e-2 gate.
fp8 DoubleRow was simulated and rejected: e4m3's 3-bit mantissa puts the
score path at ~1e-1 and the Z path at ~3e-2 absmax-rel error.

Matmul layouts (contraction on the partition dim, zero on-chip transposes):
  G[d,q]    : lhsT=A^T col-blocks [e,d-slices], rhs=xqT [e,q]  (+c per-part)
  S^T[k,q]  : lhsT=xkT [d,k-slices],  rhs=G [d,q]
  attnU^T   = exp(S^T/32 + mask_bias[k])   -- one fused ScalarE op per tile
  sumexp    : lhsT=ones [k,2], rhs=attnU^T -> [2,q]; DVE reciprocal + GpSimd
              partition-broadcast; normalize folds into the Z psum->SBUF mul
  Z^T[d,q]  : lhsT=xkN rows [k,d-slices], rhs=attnU^T [k,q]  (xkN resident)
  out[q,dv] : lhsT=Z^T [d,q-slices], rhs=WvT [d,dv]; final = psum + bv_bcast

Schedule notes (from perfetto traces; per-512-free-bf16 matmul ~216 ns warm,
LDWEIGHTS hidden -- the kernel is tensor-bound, ~119.6 us of matmul):
- Startup DMAs stay FINE-GRAINED (0.125-0.25 MB): matmuls fire as tiles land
  and the G phase tracks the ~185 GB/s startup DMA rate. Batching them into
  1 MB chunks measurably starves G.
- The ones const comes from a gpsimd memset (no DMA), so ~96 tiny warm-up
  matmuls start as soon as the queues spin up (~7 us) and the PE_HAM clock
  gate (1.2 vs 2.4 GHz) is open for the first real matmul.
- HBM bandwidth is the G-phase constraint: the later-needed streams (xs,
  xkN, WvT, bvb) are gated via dep-helpers on G psum-drain instructions so
  the G window carries only G bytes. Without this, G stalls ~6-10 us.
- Z runs in subpasses of 2 PSUM banks x 256 d-columns: with 4-bank passes
  the next pass stalls ~4 us on the previous pass's back-to-back ~0.9 us DVE
  drains (6-bank pool), and the idle PE re-colds the HAM clock.
- Phase order srow0,Z0,srow1,Z1,out0,out1 keeps DVE drains overlapped with
  the next stage's matmuls.
- Output stores split per 512-column half across the sync and scalar DMA
  queues so the final store is only 0.125 MB deep, shrinking the tail.

Queue discipline: sync carries A^T/xkN/WvT loads + output stores (dv-half 1);
scalar carries consts + xqT + xkT streams + output dv-half 0 (its only
compute is the exps); gpsimd does the ones memset, bvb load and recip
broadcasts; vector does all PSUM->SBUF drains (each fused with required
math: +c, *recip, +bv).
"""

from contextlib import ExitStack

import numpy as np
import ml_dtypes

import concourse.bacc as bacc
import concourse.mybir as mybir
import concourse.tile as tile
from concourse.bass_utils import run_bass_kernel_spmd

D = 1024       # model dim = head dim
S = 2048       # sequence length
QL = 1024      # queries per core
N_CORES = 8
SCALE = 1.0 / 32.0   # 1/sqrt(D)
MASK_NEG = -30000.0
N_WARM = 96

F32 = mybir.dt.float32
BF16 = mybir.dt.bfloat16
AF = mybir.ActivationFunctionType
BFNP = ml_dtypes.bfloat16


def _chunks(n, w):
    """[(start, width)] covering range(n) in chunks of width w."""
    return [(s, min(w, n - s)) for s in range(0, n, w)]


def _build_nc(nkt):
    kpad = nkt * 128
    nc = bacc.Bacc(None)

    atd = nc.declare_dram_parameter("atd", [8, 128, 8, 128], BF16,
                                    isOutput=False)[:]
    xqT = nc.declare_dram_parameter("xqT", [16, 128, 512], BF16,
                                    isOutput=False)[:]
    xkT = nc.declare_dram_parameter("xkT", [D, kpad], BF16, isOutput=False)[:]
    xkN = nc.declare_dram_parameter("xkN", [kpad, D], BF16, isOutput=False)[:]
    wvT = nc.declare_dram_parameter("wvT", [D, D], BF16, isOutput=False)[:]
    cT = nc.declare_dram_parameter("cT", [128, 8], F32, isOutput=False)[:]
    mbT = nc.declare_dram_parameter("mbT", [128, nkt], F32, isOutput=False)[:]
    bvb = nc.declare_dram_parameter("bvb", [128, D], F32, isOutput=False)[:]
    out_d = nc.declare_dram_parameter("out", [QL, D], BF16, isOutput=True)[:]

    with tile.TileContext(nc) as tc:
        _emit(nc, tc, nkt, atd, xqT, xkT, xkN, wvT, cT, mbT, bvb, out_d)
    nc.finalize()
    return nc


def _emit(nc, tc, nkt, atd, xqT, xkT, xkN, wvT, cT, mbT, bvb, out_d):
    with ExitStack() as ctx:
        consts = ctx.enter_context(tc.tile_pool(name="consts", bufs=1))
        # G[d,q] lives across phases 1-2.
        gpool = ctx.enter_context(tc.tile_pool(name="g", bufs=8))
        gt = [gpool.tile([128, QL], BF16, tag="gt", name=f"gt{m}")
              for m in range(8)]
        # xs (S^T lhsT stream) and xkN (Z lhsT, resident) live outside the
        # phase pools so their loads are not gated on the phase-1 release.
        xspool = ctx.enter_context(tc.tile_pool(name="xs", bufs=2))
        xknpool = ctx.enter_context(tc.tile_pool(name="xkn", bufs=1))
        # One PSUM pool for the whole kernel: "ps" (6 banks) serves G,
        # scores, Z and out; "ps_sum" (2 banks) serves warmup + sumexp.
        pps = ctx.enter_context(tc.tile_pool(name="ps", bufs=6, space="PSUM"))

        # ---------------- Phase 1: G = A @ xq^T + c ----------------
        with tc.tile_pool(name="proj", bufs=1) as pp:
            # First-matmul gating tile goes out on the scalar queue first.
            xq = [[None] * 8 for _ in range(2)]
            xq_dmas = []
            g_drains = []

            def load_xq(qc, ec, eng):
                x = pp.tile([128, 512], BF16, tag="xq", bufs=16,
                            name=f"xq{qc}_{ec}")
                di = eng.dma_start(out=x, in_=xqT[qc * 8 + ec])
                xq[qc][ec] = x
                xq_dmas.append(di)

            # ones const via engine memset: no DMA dependency, so the
            # warm-up matmuls start as soon as the queues spin up.
            ones_sb = consts.tile([128, 2], BF16, tag="ones", name="ones_sb")
            nc.gpsimd.memset(ones_sb, 1.0)
            load_xq(0, 0, nc.scalar)
            cT_sb = consts.tile([128, 8], F32, tag="cT", name="cT_sb")
            nc.scalar.dma_start(out=cT_sb, in_=cT)
            mb_sb = consts.tile([128, nkt], F32, tag="mb", name="mb_sb")
            nc.scalar.dma_start(out=mb_sb, in_=mbT)
            for ec in range(1, 8):
                load_xq(0, ec, nc.scalar)

            for ec in range(8):
                load_xq(1, ec, nc.scalar)

            # A^T d-column blocks, host-pre-blocked to exact tile layout so
            # each load is one linear DRAM burst (the previous strided
            # rearrange read 256-byte runs and paced the whole G phase).
            atw = []
            for dt in range(8):
                w = pp.tile([128, 8, 128], BF16, tag="atw", bufs=8,
                            name=f"atw{dt}")
                nc.sync.dma_start(out=w, in_=atd[dt])
                atw.append(w)

            # Tiny matmuls during the startup DMA window keep the PE busy so
            # the HAM clock gate opens before the first real matmul.
            warm_ps = pps.tile([2, 2], F32, tag="ps_sum", bufs=2,
                               name="warm_ps")
            for _ in range(N_WARM):
                nc.tensor.matmul(warm_ps, ones_sb, ones_sb,
                                 start=True, stop=True)

            for qc in range(2):
                for dt in range(8):
                    ps = pps.tile([128, 512], F32, tag="ps",
                                  name=f"psg{qc}_{dt}")
                    for ec in range(8):
                        nc.tensor.matmul(
                            ps, atw[dt][:, ec, :], xq[qc][ec],
                            start=(ec == 0), stop=(ec == 7))
                    gd = nc.vector.tensor_scalar_add(
                        gt[dt][:, qc * 512:(qc + 1) * 512], ps,
                        cT_sb[:, dt:dt + 1])
                    g_drains.append(gd)

        # ---------------- Phase 2: attention ----------------
        with tc.tile_pool(name="att", bufs=1) as at_p:
            bvb_sb = at_p.tile([128, D], F32, tag="bvb", bufs=1, name="bvb_sb")
            di = nc.gpsimd.dma_start(out=bvb_sb, in_=bvb)
            tile.add_dep_helper(di.ins, g_drains[-1].ins,
                                reason="bvb stream after G window")
            # x_keys rows resident for Z (used by both q-chunks), one DMA.
            xkn_t = xknpool.tile([128, nkt, D], BF16, tag="xkn", name="xkn_t")
            di = nc.sync.dma_start(
                out=xkn_t, in_=xkN.rearrange("(a p) d -> p a d", p=128))
            tile.add_dep_helper(di.ins, g_drains[-1].ins,
                                reason="xkN stream after G window")
            # Wv^T resident for the final out-matmul (one 2 MB DMA).
            wvb = at_p.tile([128, 8, D], BF16, tag="wv", bufs=1, name="wvb")
            di = nc.sync.dma_start(
                out=wvb, in_=wvT.rearrange("(a p) d -> p a d", p=128))

            # Preload the exp table set before the first real activation.
            warm_act = consts.tile([128, 2], F32, tag="warm_act",
                                   name="warm_act")
            nc.scalar.activation(warm_act, ones_sb, AF.Exp)

            # ---- S^T[k,q] = xkT.T @ G -> fused mask+exp, both q-chunks ----
            xs_ch = {}
            for ci, (s0, w_) in enumerate(_chunks(nkt, 3)):
                xs = xspool.tile([128, 8, w_ * 128], BF16, tag="xs",
                                 name=f"xs{ci}")
                di = nc.scalar.dma_start(
                    out=xs,
                    in_=xkT[:, s0 * 128:(s0 + w_) * 128]
                    .rearrange("(a p) s -> p a s", p=128))
                if ci == 0:
                    tile.add_dep_helper(di.ins, g_drains[4].ins,
                                        reason="xs lands just before S^T")
                for lk in range(w_):
                    xs_ch[s0 + lk] = (xs, lk)

            at = [[], []]
            for kt in range(nkt):
                xs, lk = xs_ch[kt]
                for qc in range(2):
                    ps = pps.tile([128, 512], F32, tag="ps",
                                  name=f"pss{qc}_{kt}")
                    for dc in range(8):
                        nc.tensor.matmul(
                            ps, xs[:, dc, lk * 128:(lk + 1) * 128],
                            gt[dc][:, qc * 512:(qc + 1) * 512],
                            start=(dc == 0), stop=(dc == 7))
                    a = at_p.tile([128, 512], BF16, tag="at", bufs=2 * nkt,
                                  name=f"at{qc}_{kt}")
                    nc.scalar.activation(
                        a, ps, AF.Exp,
                        bias=mb_sb[:, kt:kt + 1], scale=SCALE)
                    at[qc].append(a)

            # ---- sumexp + Z for both q-chunks, then the out-projections:
            # DVE drains of Z(qc) overlap the matmuls of the next stage. ----
            zt = [[], []]
            for qc in range(2):
                srow = pps.tile([2, 512], F32, tag="ps_sum", bufs=2,
                                name=f"srow{qc}")
                for kt in range(nkt):
                    nc.tensor.matmul(
                        srow, ones_sb, at[qc][kt],
                        start=(kt == 0), stop=(kt == nkt - 1))
                rrow = at_p.tile([2, 512], F32, tag="rrow", bufs=2,
                                 name=f"rrow{qc}")
                nc.vector.reciprocal(rrow, srow)
                rb = at_p.tile([128, 512], F32, tag="rb", bufs=2,
                               name=f"rb{qc}")
                nc.gpsimd.partition_broadcast(rb, rrow[0:1, :], channels=128)

                # Z in subpasses of 2 PSUM banks (256 d-columns each) so the
                # DVE drains recycle pool slots without stalling the PE.
                for sp in range(4):
                    pzs = [pps.tile([128, 512], F32, tag="ps",
                                    name=f"psz{qc}_{sp}_{j}")
                           for j in range(2)]
                    for kt in range(nkt):
                        for j in range(2):
                            dcol = sp * 256 + j * 128
                            nc.tensor.matmul(
                                pzs[j],
                                xkn_t[:, kt, dcol:dcol + 128],
                                at[qc][kt],
                                start=(kt == 0), stop=(kt == nkt - 1))
                    for j in range(2):
                        z = at_p.tile([128, 512], BF16, tag="zt", bufs=16,
                                      name=f"zt{qc}_{sp}_{j}")
                        nc.vector.tensor_mul(z, pzs[j], rb)
                        zt[qc].append(z)

            for qc in range(2):
                for qs in range(4):
                    o = at_p.tile([128, D], BF16, tag="o", bufs=4,
                                  name=f"o{qc}_{qs}")
                    row = (qc * 4 + qs) * 128
                    for dvc in range(2):
                        ps = pps.tile([128, 512], F32, tag="ps",
                                      name=f"pso{qc}_{qs}_{dvc}")
                        for dt in range(8):
                            nc.tensor.matmul(
                                ps, zt[qc][dt][:, qs * 128:(qs + 1) * 128],
                                wvb[:, dt, dvc * 512:(dvc + 1) * 512],
                                start=(dt == 0), stop=(dt == 7))
                        nc.vector.tensor_add(
                            o[:, dvc * 512:(dvc + 1) * 512], ps,
                            bvb_sb[:, dvc * 512:(dvc + 1) * 512])
                        # Halves ride different DMA queues so the final
                        # store is only 0.25 MB deep.
                        eng = nc.scalar if dvc == 0 else nc.sync
                        eng.dma_start(
                            out=out_d[row:row + 128,
                                      dvc * 512:(dvc + 1) * 512],
                            in_=o[:, dvc * 512:(dvc + 1) * 512])


def _prep_inputs(x, mask, Wq, bq, Wk, bk, Wv, bv):
    x = np.asarray(x, dtype=np.float32)
    mask = np.asarray(mask, dtype=bool)
    Wq = np.asarray(Wq, dtype=np.float64)
    bq = np.asarray(bq, dtype=np.float64)
    Wk = np.asarray(Wk, dtype=np.float64)
    Wv = np.asarray(Wv, dtype=np.float32)
    bv = np.asarray(bv, dtype=np.float32)
    del bk  # exactly cancelled by softmax shift invariance

    # Host weight folding (data-independent): A^T = Wq^T Wk, c = Wk^T bq.
    # A^T is pre-blocked to the device tile layout [dt][p, e-block, d-col]
    # so each 0.25 MB load is a single linear DRAM burst.
    at_f = (Wq.T @ Wk).astype(BFNP)
    at_h = np.ascontiguousarray(np.stack(
        [at_f[:, dt * 128:(dt + 1) * 128].reshape(8, 128, 128)
         .transpose(1, 0, 2) for dt in range(8)]))
    c = (Wk.T @ bq).astype(np.float32)
    cT_h = np.ascontiguousarray(c.reshape(8, 128).T)
    wvT_h = np.ascontiguousarray(Wv.T.astype(BFNP))
    bvb_h = np.ascontiguousarray(np.broadcast_to(bv, (128, D)))

    cnts = [int(np.flatnonzero(mask[b]).size) for b in range(4)]
    nkt = max(1, int(np.ceil(max(cnts) / 128)))
    kpad = nkt * 128

    xkn_b, xkt_b, mbt_b = [], [], []
    for b in range(4):
        idx = np.flatnonzero(mask[b])
        xk = np.zeros((kpad, D), dtype=BFNP)
        xk[:len(idx)] = x[b, idx].astype(BFNP)
        xkn_b.append(np.ascontiguousarray(xk))
        xkt_b.append(np.ascontiguousarray(xk.T))
        mb = np.where(np.arange(kpad) < len(idx), 0.0,
                      MASK_NEG).astype(np.float32)
        mbt_b.append(np.ascontiguousarray(mb.reshape(nkt, 128).T))

    in_maps = []
    for c_i in range(N_CORES):
        b, h = divmod(c_i, 2)
        xq_f = x[b, h * QL:(h + 1) * QL, :].T.astype(BFNP)
        xqT_c = np.ascontiguousarray(np.stack(
            [xq_f[ec * 128:(ec + 1) * 128, qc * 512:(qc + 1) * 512]
             for qc in range(2) for ec in range(8)]))
        in_maps.append({
            "atd": at_h, "xqT": xqT_c, "xkT": xkt_b[b], "xkN": xkn_b[b],
            "wvT": wvT_h, "cT": cT_h, "mbT": mbt_b[b], "bvb": bvb_h,
        })
    return in_maps, nkt


def run(x, mask, Wq, bq, Wk, bk, Wv, bv, trace=False):
    """Build + run; returns (output, BassKernelResults)."""
    in_maps, nkt = _prep_inputs(x, mask, Wq, bq, Wk, bk, Wv, bv)
    nc = _build_nc(nkt)
    res = run_bass_kernel_spmd(nc, in_maps, list(range(N_CORES)), trace=trace)
    out = np.empty((4, S, D), dtype=np.float32)
    for c_i in range(N_CORES):
        b, h = divmod(c_i, 2)
        out[b, h * QL:(h + 1) * QL, :] = np.asarray(
            res.results[c_i]["out"]).astype(np.float32)
    return out, res


def kernel(x, mask, Wq, bq, Wk, bk, Wv, bv):
    out, _ = run(x, mask, Wq, bq, Wk, bk, Wv, bv)
    return out
